# revision 11
# baseline (speedup 1.0000x reference)
"""CrossAttentionBlock Trainium2 kernel (v5).

Math (reference):
    q = Wq@xq + bq; k = Wk@xkv (bk dropped: softmax-invariant);
    S = (q^T k)/4; P = softmax_rows(S); out = (Wv@xkv + bv) @ P^T;
    y = x_q + gamma*out

Kernel strategy (8 cores, sequence-parallel over the N=13824 queries; each
core owns NQ=1728 queries against full K/V):
  * The k path never materializes: S^T = k^T q = xkv^T (Wk^T q), so the
    host folds Wqk = scale*Wk^T Wq (and bqk) and the kernel projects xq
    ONCE to qk = Wqk@xq + bqk [C, NQ] via float32r matmuls.  S^T tiles
    contract xkv (stationary, fp8 DoubleRow) against qk (moving, fp8
    DoubleRow) over the 128 channels.
  * x_kv arrives host-packed in fp8 DoubleRow layout [64, 2N]; the v^T
    projection (stationary xkv tile, moving Wv^T) runs fp8+DoubleRow;
    gamma folds into its PSUM evacuation; gamma*bv + x_q folds into the
    residual on the host.  v^T slots are interleaved into chunk 0's pair
    loop so their evacuations fill the engines' head time.
  * exp of 23.9M elem/core is the hard limit; only ScalarE (true exp,
    ~905ns/864elem) and VectorE (Schraudolph int8 bit-trick, ~1025ns) can
    read PSUM, so exp strictly alternates, with ScalarE taking the pairs
    where VectorE runs the deferred epilogue (reciprocal / normalize).
  * The pair pipeline is SEAMLESS across chunks: each chunk's loop body
    carries the previous chunk's last 7 out_u/rowsum pairs (iterations
    0-6), runs the previous chunk's epilogue at iterations 7/10 (when its
    rowsum has just completed), and starts its own out_u at iteration 12
    with a 2-per-iteration catch-up — so the single-buffered out_u/rowsum
    PSUM banks are always free exactly when needed and no engine sees a
    chunk-boundary bubble.
  * Deferred normalization: exp(S^T) feeds accumulating out_u and
    ones-row (rowsum) matmuls; per chunk 1/rowsum broadcasts via a 1->128
    ones matmul, VectorE multiplies out_u (PSUM) by it, GpSimd adds the
    residual.  The final chunk's epilogue runs as two independent
    half-width chains to shorten the drain.
  * Attention contributes O(1e-4) of the output, so fp8 quantization on
    the attention path is invisible; the residual stays fp32.
"""

import contextlib

import numpy as np

import concourse.bass as bass
import concourse.mybir as mybir
from concourse import bacc
from concourse.tile import TileContext
from concourse.bass_utils import run_bass_kernel_spmd

F32 = mybir.dt.float32
F32R = mybir.dt.float32r
FP8 = mybir.dt.float8e4
I8 = mybir.dt.int8
AF = mybir.ActivationFunctionType
DR = mybir.MatmulPerfMode.DoubleRow
ADD = mybir.AluOpType.add
MUL = mybir.AluOpType.mult

C = 128           # channels
RC = 16           # reduced (q/k) channels
KD = 64           # DoubleRow partition dim for the xkv contraction
D = H = W = 24
N = D * H * W     # 13824 tokens
NCORES = 8
NQ = N // NCORES  # 1728 queries per core
CHUNK = 432       # query chunk
NCHUNKS = NQ // CHUNK   # 4
MT = N // 128     # 108 key tiles
PAIRS = MT // 2   # 54 key-tile pairs per chunk
LAGP = 7          # steady-state out/rs lag behind exp (pairs)
EXB = 16          # ex-tile ring depth (max transient lag is 13)
QTR = N // 4      # xkv streams in 4 column-quarters

LOG2E = 1.4426950408889634
EXP8_SCALE = 8.0 * LOG2E      # e4m3: 3 mantissa bits, bias 7
EXP8_BIAS = 56.0 - 0.3        # 7*8 + Schraudolph offset

# v^T evacuation engine per slot e (0=ScalarE 1=VectorE): 8 ACT / 6 DVE
VEVAC_PAT = [0, 1, 0, 1, 0, 1, 0, 1, 0, 1, 0, 1, 0, 0]


def _chunk_pattern(ch):
    """Exp engine per pair: alternate ACT/DVE; ACT additionally takes the
    pairs where DVE runs the previous chunk's epilogue (7: reciprocal,
    11: normalize-multiply).  Chunk 0 has no pending epilogue; chunk 3
    gives ACT the final pair so DVE is free at the drain."""
    pat = [0 if s % 2 == 0 else 1 for s in range(PAIRS)]
    if ch == 0:
        pat[27] = 0
    else:
        pat[7] = 0
        pat[11] = 0
    if ch == NCHUNKS - 1:
        pat[53] = 0
    return pat


_BUILD_CACHE: dict = {}


def build_nc(repeats: int = 1):
    key = repeats
    if key in _BUILD_CACHE:
        return _BUILD_CACHE[key]

    nc = bacc.Bacc("TRN2", target_bir_lowering=False, debug=False,
                   num_devices=NCORES)
    xq = nc.dram_tensor("xq", [C, NQ], F32R, kind="ExternalInput").ap()
    xkvd = nc.dram_tensor("xkvd", [KD, 2 * N], FP8, kind="ExternalInput").ap()
    # wblob packs wqkT [C,128] + bqk [C,1] + gam [C,1] (f32 bits)
    wblob = nc.dram_tensor("wblob", [C, C + 2], F32R, kind="ExternalInput").ap()
    wvd = nc.dram_tensor("wvd", [KD, 2 * C], FP8, kind="ExternalInput").ap()
    xkvp = nc.dram_tensor("xkvp", [C, 1536], FP8, kind="ExternalInput").ap()
    y = nc.dram_tensor("y", [C, NQ], F32, kind="ExternalOutput").ap()

    with TileContext(nc) as tc, contextlib.ExitStack() as ctx:
        cpool = ctx.enter_context(tc.tile_pool(name="consts", bufs=1))
        ppool = ctx.enter_context(tc.tile_pool(name="psum", bufs=1, space="PSUM"))
        spool = ctx.enter_context(tc.tile_pool(name="work", bufs=1))

        # ---- input DMAs.  HWDGE desc-gen (~630ns) and the DMA device are
        # both globally serialized, so: few DMAs, critical-path first.
        wblob_sb = cpool.tile([C, C + 2], F32R)
        nc.sync.dma_start(wblob_sb[:], wblob[:])
        wqkT_sb = wblob_sb[:, 0:C]
        bqk_sb = wblob_sb[:, C:C + 1].bitcast(F32)
        gam_sb = wblob_sb[:, C + 1:C + 2].bitcast(F32)

        xq_sb = cpool.tile([C, NQ], F32R)
        xkvd_sb = cpool.tile([KD, 2 * N], FP8)
        wvd_sb = cpool.tile([KD, 2 * C], FP8)
        xkv3 = xkvd_sb.rearrange("p (o x) -> p o x", o=2)
        xkvd3 = xkvd.rearrange("p (o x) -> p o x", o=2)

        def xkv_qtr(j):
            nc.sync.dma_start(xkv3[:, :, j * QTR:(j + 1) * QTR],
                              xkvd3[:, :, j * QTR:(j + 1) * QTR])

        xkvp_sb = cpool.tile([C, 1536], FP8)
        xkv_qtr(0)
        nc.sync.dma_start(xq_sb[:, 0:CHUNK], xq[:, 0:CHUNK])
        nc.sync.dma_start(wvd_sb[:], wvd[:])
        nc.sync.dma_start(xkvp_sb[:], xkvp[:])
        wv3 = wvd_sb.rearrange("p (o c) -> p o c", o=2)

        ones_db = cpool.tile([C, 32], FP8)   # lhsT for DoubleRow rowsum
        nc.gpsimd.memset(ones_db[:], 1.0)
        ones_row = cpool.tile([1, C], F32)   # lhsT for 1->128 broadcast matmul
        nc.gpsimd.memset(ones_row[:], 1.0)

        def slot_st():
            return ppool.tile([C, 1024], F32, tag="st", bufs=3, name="pslot")

        # ---- qk projection: qk = Wqk@xq + bqk (float32r), fp8 out; each
        # evacuated 864-col half immediately unlocks its DoubleRow remap.
        qk_sp = cpool.tile([C, NQ], FP8)
        qk_db = cpool.tile([KD, 2 * NQ], FP8)

        qk3 = qk_db.rearrange("p (o x) -> p o x", o=2)

        def qk_chunk(i):
            stq = slot_st()
            sl = bass.ts(i, CHUNK)
            nc.tensor.matmul(stq[:, 0:CHUNK], wqkT_sb, xq_sb[:, sl],
                             start=True, stop=True)
            d2 = qk_sp[:, sl]
            nc.scalar.activation(d2, stq[:, 0:CHUNK], AF.Identity, bias=bqk_sb)
            # qk_sp partition 2p+o holds channel p+64o (host-permuted Wqk),
            # so one DMA writes the DoubleRow layout directly
            nc.sync.dma_start(qk3[:, :, sl], qk_sp[:, sl])

        # warm the PE pstate with a trivial matmul before the first real one
        warm = ppool.tile([C, 512], F32, tag="outu", bufs=1, name="warm")
        nc.tensor.matmul(warm[0:C, 0:1], ones_row[:], ones_row[:, 0:1],
                         start=True, stop=True)
        qk_chunk(0)
        # remaining inputs dispatch AFTER chunk 0's qk remap (SP in-order)
        xkv_qtr(1)
        nc.sync.dma_start(xq_sb[:, CHUNK:3 * CHUNK], xq[:, CHUNK:3 * CHUNK])
        xkv_qtr(2)
        nc.sync.dma_start(xq_sb[:, 3 * CHUNK:NQ], xq[:, 3 * CHUNK:NQ])
        xkv_qtr(3)

        vt_sb = cpool.tile([C, N], FP8)

        def v_slot(e):
            """v^T projection slot e (tiles 8e..8e+7; slot 13 has 4),
            evacuated with gamma folded in, on the patterned engine."""
            psv = slot_st()
            nt = 4 if e == 13 else 8
            for j in range(nt):
                t = 8 * e + j
                nc.tensor.matmul(psv[:, bass.ts(j, 128)],
                                 xkv3[:, :, bass.ts(t, 128)], wv3[:],
                                 start=True, stop=True, perf_mode=DR)
            dst = vt_sb[:, bass.ds(1024 * e, 128 * nt)]
            src = psv[:, 0:128 * nt]
            if VEVAC_PAT[e] == 0:
                nc.scalar.activation(dst, src, AF.Identity, scale=gam_sb)
            else:
                nc.vector.tensor_scalar(out=dst, in0=src, scalar1=gam_sb,
                                        scalar2=None, op0=MUL)

        # ---- attention: one seamless pair pipeline ----------------------
        pend = {}
        live = {}   # per-chunk outu/rs psum tiles + ex ring

        def epi_a():
            pend["recip"] = recip = spool.tile([1, CHUNK], F32, tag="recip",
                                               bufs=2, name="recip")
            nc.vector.reciprocal_approx_fast(out=recip[:], in_=pend.pop("rs")[:])

        def epi_b():
            sl = pend.pop("sl")
            bcpt = ppool.tile([C, 1024], F32, tag="st", bufs=3, name="bcpt")
            bcp = bcpt[:, 0:CHUNK]
            nc.tensor.matmul(bcp, ones_row[:], pend.pop("recip")[:],
                             start=True, stop=True)
            bcs = spool.tile([C, CHUNK], F32, tag="bcs", bufs=2)
            nc.scalar.copy(bcs[:], bcp)
            t1 = spool.tile([C, CHUNK], F32, tag="t1", bufs=2)
            nc.vector.tensor_mul(t1[:], pend.pop("outu")[:], bcs[:])
            res = spool.tile([C, CHUNK], F32, tag="res", bufs=2)
            nc.gpsimd.tensor_add(res[:], t1[:], xq_sb[:, sl].bitcast(F32))
            nc.sync.dma_start(y[:, sl], res[:])

        def epi_final():
            sl0 = pend.pop("sl").start
            epi_a()
            recip = pend.pop("recip")
            outu = pend.pop("outu")
            pieces = [(0, 304), (304, 128)]
            for h, (off, w) in enumerate(pieces):
                hs = bass.ds(off, w)
                bcpt = ppool.tile([C, 1024], F32, tag="st", bufs=3, name="bcpt")
                bcp = bcpt[:, 0:w]
                nc.tensor.matmul(bcp, ones_row[:], recip[:, hs],
                                 start=True, stop=True)
                bcs = spool.tile([C, w], F32, tag="bcs2", bufs=3, name="bcs")
                nc.scalar.copy(bcs[:], bcp)
                t1 = spool.tile([C, w], F32, tag="t12", bufs=3, name="t1")
                nc.vector.tensor_mul(t1[:], outu[:, hs], bcs[:])
                res = spool.tile([C, w], F32, tag="res2", bufs=3, name="res")
                xs = xq_sb[:, bass.ds(sl0 + off, w)].bitcast(F32)
                if h == 0:
                    nc.gpsimd.tensor_add(res[:], t1[:], xs)
                else:
                    nc.vector.tensor_add(res[:], t1[:], xs)
                nc.sync.dma_start(y[:, bass.ds(sl0 + off, w)], res[:])

        def emit_st(ch, s, pat, plain=False):
            """S^T pair s of chunk ch + its exp.  plain=True uses the
            non-DoubleRow copy of the first xkv tiles so chunk 0 can start
            before the qk DoubleRow remap DMA lands."""
            stp = ppool.tile([C, 1024], F32, tag="st", bufs=3)
            for j in range(2):
                t = 2 * s + j
                if plain:
                    nc.tensor.matmul(stp[:, 512 * j:512 * j + CHUNK],
                                     xkvp_sb[:, bass.ts(t, 128)],
                                     qk_sp[:, bass.ts(ch, CHUNK)],
                                     start=True, stop=True)
                else:
                    nc.tensor.matmul(stp[:, 512 * j:512 * j + CHUNK],
                                     xkv3[:, :, bass.ts(t, 128)],
                                     qk3[:, :, bass.ts(ch, CHUNK)],
                                     start=True, stop=True, perf_mode=DR)
            st3 = stp.rearrange("p (b x) -> p b x", b=2)[:, :, 0:CHUNK]
            ex = spool.tile([C, 2 * CHUNK], FP8, tag="ex", bufs=EXB)
            ex3 = ex.rearrange("p (b x) -> p b x", b=2)
            if pat[s] == 0:
                nc.scalar.activation(ex3, st3, AF.Exp)
            else:
                nc.vector.tensor_scalar(out=ex3.bitcast(I8), in0=st3,
                                        scalar1=EXP8_SCALE, scalar2=EXP8_BIAS,
                                        op0=MUL, op1=ADD)
            live.setdefault("ex", {})[(ch, s)] = ex

        def emit_ou(ch, s):
            """out_u/rowsum pair s of chunk ch (consumes its ex tile)."""
            ex = live["ex"].pop((ch, s))
            ex3 = ex.rearrange("p (b x) -> p b x", b=2)
            vt3 = vt_sb[:, bass.ds(256 * s, 256)].rearrange("p (b c) -> p b c", b=2)
            nc.tensor.matmul(live["outu"], vt3, ex3, perf_mode=DR,
                             start=(s == 0), stop=(s == PAIRS - 1))
            o3 = ones_db.rearrange("p (b c) -> p b c", b=2)[:, :, 0:1]
            nc.tensor.matmul(live["rs"], o3, ex3, perf_mode=DR,
                             start=(s == 0), stop=(s == PAIRS - 1))
            if s == PAIRS - 1:
                pend.update(outu=live.pop("outu"), rs=live.pop("rs"),
                            sl=bass.ts(ch, CHUNK))

        for e in range(6):
            v_slot(e)

        for rep in range(repeats):
            for ch in range(NCHUNKS):
                first = ch == 0 and rep == 0
                pat = _chunk_pattern(ch)
                for up in range(PAIRS):
                    if up == 7 and "rs" in pend:
                        epi_a()
                    if up == 10 and "recip" in pend:
                        epi_b()
                    if first and up % 3 == 0 and 6 + up // 3 < 14:
                        v_slot(6 + up // 3)
                    if first and up in (24, 28, 32):
                        qk_chunk(1 + (up - 24) // 4)
                    if up == 12:
                        live["outu"] = ppool.tile([C, CHUNK], F32, tag="outu",
                                                  name="outu")
                        live["rs"] = ppool.tile([1, CHUNK], F32, tag="rs",
                                                name="rs")
                    emit_st(ch, up, pat, plain=(first and up < 6))
                    if not first:
                        if up < LAGP:
                            emit_ou(ch - 1, 47 + up)
                    if 12 <= up < 17:
                        emit_ou(ch, 2 * (up - 12))
                        emit_ou(ch, 2 * (up - 12) + 1)
                    elif up >= 17:
                        emit_ou(ch, up - LAGP)
            for p in range(47, PAIRS):          # final chunk's tail
                emit_ou(NCHUNKS - 1, p)
            if rep != repeats - 1:
                epi_a()
                epi_b()
                tc.strict_bb_all_engine_barrier()
        if "rs" in pend:
            epi_final()

    nc.compile()
    _BUILD_CACHE[key] = nc
    return nc


def _pack_db(a):
    """[128, X] -> DoubleRow [64, 2X] (virtual row r = p + 64*o)."""
    x = a.shape[1]
    return np.ascontiguousarray(a.reshape(2, KD, x).transpose(1, 0, 2)
                                .reshape(KD, 2 * x))


def _prep_in_maps(x_q, x_kv, Wq, bq, Wk, bk, Wv, bv, gamma):
    f32 = np.float32
    f8 = mybir.dt.np(FP8)
    x_q = np.asarray(x_q, f32).reshape(C, N)
    x_kv = np.asarray(x_kv, f32).reshape(C, N)
    Wq = np.asarray(Wq, f32)
    bq = np.asarray(bq, f32)
    Wk = np.asarray(Wk, f32)
    Wv = np.asarray(Wv, f32)
    bv = np.asarray(bv, f32)
    gamma = float(np.asarray(gamma, f32).reshape(()))

    scale = 1.0 / np.sqrt(np.float32(RC))
    xkvd = _pack_db(x_kv).astype(f8)
    wvd = _pack_db(np.ascontiguousarray(Wv.T)).astype(f8)
    # S^T = xkv^T qk with qk = scale*(Wk^T Wq xq + Wk^T bq); bk dropped
    # (softmax-invariant per-query constant).
    # qk output channel at partition j is perm[j] = j//2 + 64*(j%2), so the
    # DoubleRow remap [64,2,x] <- [128,x] is a single in-order DMA
    perm = np.arange(C) // 2 + KD * (np.arange(C) % 2)
    wblob = np.empty((C, C + 2), f32)
    wblob[:, 0:C] = (scale * (Wq.T @ Wk))[:, perm]   # lhsT [ch, c']
    wblob[:, C] = (scale * (Wk.T @ bq))[perm]
    wblob[:, C + 1] = gamma
    xkvp = np.ascontiguousarray(x_kv[perm, 0:1536]).astype(f8)
    resid_bias = (gamma * bv).astype(f32)  # softmax rows sum to 1

    in_maps = []
    for c in range(NCORES):
        xq_slice = np.ascontiguousarray(
            x_q[:, c * NQ:(c + 1) * NQ] + resid_bias[:, None], f32)
        in_maps.append({
            "xq": xq_slice, "xkvd": xkvd, "wblob": wblob, "wvd": wvd,
            "xkvp": xkvp,
        })
    return in_maps


def kernel(x_q, x_kv, Wq, bq, Wk, bk, Wv, bv, gamma):
    nc = build_nc(repeats=1)
    in_maps = _prep_in_maps(x_q, x_kv, Wq, bq, Wk, bk, Wv, bv, gamma)
    res = run_bass_kernel_spmd(nc, in_maps, list(range(NCORES)))
    out = np.concatenate([res.results[c]["y"] for c in range(NCORES)], axis=1)
    return out.reshape(1, C, D, H, W).astype(np.float32)


# revision 12
# speedup vs baseline: 1.0032x; 1.0032x over previous
"""CrossAttentionBlock Trainium2 kernel (v5).

Math (reference):
    q = Wq@xq + bq; k = Wk@xkv (bk dropped: softmax-invariant);
    S = (q^T k)/4; P = softmax_rows(S); out = (Wv@xkv + bv) @ P^T;
    y = x_q + gamma*out

Kernel strategy (8 cores, sequence-parallel over the N=13824 queries; each
core owns NQ=1728 queries against full K/V):
  * The k path never materializes: S^T = k^T q = xkv^T (Wk^T q), so the
    host folds Wqk = scale*Wk^T Wq (and bqk) and the kernel projects xq
    ONCE to qk = Wqk@xq + bqk [C, NQ] via float32r matmuls.  S^T tiles
    contract xkv (stationary, fp8 DoubleRow) against qk (moving, fp8
    DoubleRow) over the 128 channels.
  * x_kv arrives host-packed in fp8 DoubleRow layout [64, 2N]; the v^T
    projection (stationary xkv tile, moving Wv^T) runs fp8+DoubleRow;
    gamma folds into its PSUM evacuation; gamma*bv + x_q folds into the
    residual on the host.  v^T slots are interleaved into chunk 0's pair
    loop so their evacuations fill the engines' head time.
  * exp of 23.9M elem/core is the hard limit; only ScalarE (true exp,
    ~905ns/864elem) and VectorE (Schraudolph int8 bit-trick, ~1025ns) can
    read PSUM, so exp strictly alternates, with ScalarE taking the pairs
    where VectorE runs the deferred epilogue (reciprocal / normalize).
  * The pair pipeline is SEAMLESS across chunks: each chunk's loop body
    carries the previous chunk's last 7 out_u/rowsum pairs (iterations
    0-6), runs the previous chunk's epilogue at iterations 7/10 (when its
    rowsum has just completed), and starts its own out_u at iteration 12
    with a 2-per-iteration catch-up — so the single-buffered out_u/rowsum
    PSUM banks are always free exactly when needed and no engine sees a
    chunk-boundary bubble.
  * Deferred normalization: exp(S^T) feeds accumulating out_u and
    ones-row (rowsum) matmuls; per chunk 1/rowsum broadcasts via a 1->128
    ones matmul, VectorE multiplies out_u (PSUM) by it, GpSimd adds the
    residual.  The final chunk's epilogue runs as two independent
    half-width chains to shorten the drain.
  * Attention contributes O(1e-4) of the output, so fp8 quantization on
    the attention path is invisible; the residual stays fp32.
"""

import contextlib

import numpy as np

import concourse.bass as bass
import concourse.mybir as mybir
from concourse import bacc
from concourse.tile import TileContext
from concourse.bass_utils import run_bass_kernel_spmd

F32 = mybir.dt.float32
F32R = mybir.dt.float32r
FP8 = mybir.dt.float8e4
I8 = mybir.dt.int8
AF = mybir.ActivationFunctionType
DR = mybir.MatmulPerfMode.DoubleRow
ADD = mybir.AluOpType.add
MUL = mybir.AluOpType.mult

C = 128           # channels
RC = 16           # reduced (q/k) channels
KD = 64           # DoubleRow partition dim for the xkv contraction
D = H = W = 24
N = D * H * W     # 13824 tokens
NCORES = 8
NQ = N // NCORES  # 1728 queries per core
CHUNK = 432       # query chunk
NCHUNKS = NQ // CHUNK   # 4
MT = N // 128     # 108 key tiles
PAIRS = MT // 2   # 54 key-tile pairs per chunk
LAGP = 7          # steady-state out/rs lag behind exp (pairs)
EXB = 16          # ex-tile ring depth (max transient lag is 13)
QTR = N // 4      # xkv streams in 4 column-quarters

LOG2E = 1.4426950408889634
EXP8_SCALE = 8.0 * LOG2E      # e4m3: 3 mantissa bits, bias 7
EXP8_BIAS = 56.0 - 0.3        # 7*8 + Schraudolph offset

# v^T evacuation engine per slot e (0=ScalarE 1=VectorE): 8 ACT / 6 DVE
VEVAC_PAT = [0, 1, 0, 1, 0, 1, 0, 1, 0, 1, 0, 1, 0, 0]


def _chunk_pattern(ch):
    """Exp engine per pair: alternate ACT/DVE; ACT additionally takes the
    pairs where DVE runs the previous chunk's epilogue (7: reciprocal,
    11: normalize-multiply).  Chunk 0 has no pending epilogue; chunk 3
    gives ACT the final pair so DVE is free at the drain."""
    pat = [0 if s % 2 == 0 else 1 for s in range(PAIRS)]
    if ch == 0:
        pat[27] = 0
    else:
        pat[7] = 0
        pat[11] = 0
    if ch == NCHUNKS - 1:
        pat[53] = 0
    return pat


_BUILD_CACHE: dict = {}


def build_nc(repeats: int = 1):
    key = repeats
    if key in _BUILD_CACHE:
        return _BUILD_CACHE[key]

    nc = bacc.Bacc("TRN2", target_bir_lowering=False, debug=False,
                   num_devices=NCORES)
    xq = nc.dram_tensor("xq", [C, NQ], F32R, kind="ExternalInput").ap()
    xkvd = nc.dram_tensor("xkvd", [KD, 2 * N], FP8, kind="ExternalInput").ap()
    # wblob packs wqkT [C,128] + bqk [C,1] + gam [C,1] (f32 bits)
    wblob = nc.dram_tensor("wblob", [C, C + 2], F32R, kind="ExternalInput").ap()
    wvd = nc.dram_tensor("wvd", [KD, 2 * C], FP8, kind="ExternalInput").ap()
    xkvp = nc.dram_tensor("xkvp", [C, 1536], FP8, kind="ExternalInput").ap()
    y = nc.dram_tensor("y", [C, NQ], F32, kind="ExternalOutput").ap()

    with TileContext(nc) as tc, contextlib.ExitStack() as ctx:
        cpool = ctx.enter_context(tc.tile_pool(name="consts", bufs=1))
        ppool = ctx.enter_context(tc.tile_pool(name="psum", bufs=1, space="PSUM"))
        spool = ctx.enter_context(tc.tile_pool(name="work", bufs=1))

        # ---- input DMAs.  HWDGE desc-gen (~630ns) and the DMA device are
        # both globally serialized, so: few DMAs, critical-path first.
        wblob_sb = cpool.tile([C, C + 2], F32R)
        nc.sync.dma_start(wblob_sb[:], wblob[:])
        wqkT_sb = wblob_sb[:, 0:C]
        bqk_sb = wblob_sb[:, C:C + 1].bitcast(F32)
        gam_sb = wblob_sb[:, C + 1:C + 2].bitcast(F32)

        xq_sb = cpool.tile([C, NQ], F32R)
        xkvd_sb = cpool.tile([KD, 2 * N], FP8)
        wvd_sb = cpool.tile([KD, 2 * C], FP8)
        xkv3 = xkvd_sb.rearrange("p (o x) -> p o x", o=2)
        xkvd3 = xkvd.rearrange("p (o x) -> p o x", o=2)

        def xkv_qtr(j):
            nc.sync.dma_start(xkv3[:, :, j * QTR:(j + 1) * QTR],
                              xkvd3[:, :, j * QTR:(j + 1) * QTR])

        xkvp_sb = cpool.tile([C, 1536], FP8)
        nc.sync.dma_start(xq_sb[:, 0:CHUNK], xq[:, 0:CHUNK])
        nc.sync.dma_start(xkvp_sb[:], xkvp[:])
        xkv_qtr(0)
        nc.sync.dma_start(wvd_sb[:], wvd[:])
        wv3 = wvd_sb.rearrange("p (o c) -> p o c", o=2)

        ones_db = cpool.tile([C, 32], FP8)   # lhsT for DoubleRow rowsum
        nc.gpsimd.memset(ones_db[:], 1.0)
        ones_row = cpool.tile([1, C], F32)   # lhsT for 1->128 broadcast matmul
        nc.gpsimd.memset(ones_row[:], 1.0)

        def slot_st():
            return ppool.tile([C, 1024], F32, tag="st", bufs=3, name="pslot")

        # ---- qk projection: qk = Wqk@xq + bqk (float32r), fp8 out; each
        # evacuated 864-col half immediately unlocks its DoubleRow remap.
        qk_sp = cpool.tile([C, NQ], FP8)
        qk_db = cpool.tile([KD, 2 * NQ], FP8)

        qk3 = qk_db.rearrange("p (o x) -> p o x", o=2)

        def qk_chunk(i):
            stq = slot_st()
            sl = bass.ts(i, CHUNK)
            nc.tensor.matmul(stq[:, 0:CHUNK], wqkT_sb, xq_sb[:, sl],
                             start=True, stop=True)
            d2 = qk_sp[:, sl]
            nc.scalar.activation(d2, stq[:, 0:CHUNK], AF.Identity, bias=bqk_sb)
            # qk_sp partition 2p+o holds channel p+64o (host-permuted Wqk),
            # so one DMA writes the DoubleRow layout directly
            nc.sync.dma_start(qk3[:, :, sl], qk_sp[:, sl])

        # warm the PE pstate with a trivial matmul before the first real one
        warm = ppool.tile([C, 512], F32, tag="outu", bufs=1, name="warm")
        nc.tensor.matmul(warm[0:C, 0:1], ones_row[:], ones_row[:, 0:1],
                         start=True, stop=True)
        qk_chunk(0)
        # remaining inputs dispatch AFTER chunk 0's qk remap (SP in-order)
        xkv_qtr(1)
        nc.sync.dma_start(xq_sb[:, CHUNK:3 * CHUNK], xq[:, CHUNK:3 * CHUNK])
        xkv_qtr(2)
        nc.sync.dma_start(xq_sb[:, 3 * CHUNK:NQ], xq[:, 3 * CHUNK:NQ])
        xkv_qtr(3)

        vt_sb = cpool.tile([C, N], FP8)

        def v_slot(e):
            """v^T projection slot e (tiles 8e..8e+7; slot 13 has 4),
            evacuated with gamma folded in, on the patterned engine."""
            psv = slot_st()
            nt = 4 if e == 13 else 8
            for j in range(nt):
                t = 8 * e + j
                nc.tensor.matmul(psv[:, bass.ts(j, 128)],
                                 xkv3[:, :, bass.ts(t, 128)], wv3[:],
                                 start=True, stop=True, perf_mode=DR)
            dst = vt_sb[:, bass.ds(1024 * e, 128 * nt)]
            src = psv[:, 0:128 * nt]
            if VEVAC_PAT[e] == 0:
                nc.scalar.activation(dst, src, AF.Identity, scale=gam_sb)
            else:
                nc.vector.tensor_scalar(out=dst, in0=src, scalar1=gam_sb,
                                        scalar2=None, op0=MUL)

        # ---- attention: one seamless pair pipeline ----------------------
        pend = {}
        live = {}   # per-chunk outu/rs psum tiles + ex ring

        def epi_a():
            pend["recip"] = recip = spool.tile([1, CHUNK], F32, tag="recip",
                                               bufs=2, name="recip")
            nc.vector.reciprocal_approx_fast(out=recip[:], in_=pend.pop("rs")[:])

        def epi_b():
            sl = pend.pop("sl")
            bcpt = ppool.tile([C, 1024], F32, tag="st", bufs=3, name="bcpt")
            bcp = bcpt[:, 0:CHUNK]
            nc.tensor.matmul(bcp, ones_row[:], pend.pop("recip")[:],
                             start=True, stop=True)
            bcs = spool.tile([C, CHUNK], F32, tag="bcs", bufs=2)
            nc.scalar.copy(bcs[:], bcp)
            t1 = spool.tile([C, CHUNK], F32, tag="t1", bufs=2)
            nc.vector.tensor_mul(t1[:], pend.pop("outu")[:], bcs[:])
            res = spool.tile([C, CHUNK], F32, tag="res", bufs=2)
            nc.gpsimd.tensor_add(res[:], t1[:], xq_sb[:, sl].bitcast(F32))
            nc.sync.dma_start(y[:, sl], res[:])

        def epi_final():
            sl0 = pend.pop("sl").start
            epi_a()
            recip = pend.pop("recip")
            outu = pend.pop("outu")
            pieces = [(0, 304), (304, 128)]
            for h, (off, w) in enumerate(pieces):
                hs = bass.ds(off, w)
                bcpt = ppool.tile([C, 1024], F32, tag="st", bufs=3, name="bcpt")
                bcp = bcpt[:, 0:w]
                nc.tensor.matmul(bcp, ones_row[:], recip[:, hs],
                                 start=True, stop=True)
                bcs = spool.tile([C, w], F32, tag="bcs2", bufs=3, name="bcs")
                nc.scalar.copy(bcs[:], bcp)
                t1 = spool.tile([C, w], F32, tag="t12", bufs=3, name="t1")
                nc.vector.tensor_mul(t1[:], outu[:, hs], bcs[:])
                res = spool.tile([C, w], F32, tag="res2", bufs=3, name="res")
                xs = xq_sb[:, bass.ds(sl0 + off, w)].bitcast(F32)
                if h == 0:
                    nc.gpsimd.tensor_add(res[:], t1[:], xs)
                else:
                    nc.vector.tensor_add(res[:], t1[:], xs)
                nc.sync.dma_start(y[:, bass.ds(sl0 + off, w)], res[:])

        def emit_st(ch, s, pat, plain=False):
            """S^T pair s of chunk ch + its exp.  plain=True uses the
            non-DoubleRow copy of the first xkv tiles so chunk 0 can start
            before the qk DoubleRow remap DMA lands."""
            stp = ppool.tile([C, 1024], F32, tag="st", bufs=3)
            for j in range(2):
                t = 2 * s + j
                if plain:
                    nc.tensor.matmul(stp[:, 512 * j:512 * j + CHUNK],
                                     xkvp_sb[:, bass.ts(t, 128)],
                                     qk_sp[:, bass.ts(ch, CHUNK)],
                                     start=True, stop=True)
                else:
                    nc.tensor.matmul(stp[:, 512 * j:512 * j + CHUNK],
                                     xkv3[:, :, bass.ts(t, 128)],
                                     qk3[:, :, bass.ts(ch, CHUNK)],
                                     start=True, stop=True, perf_mode=DR)
            st3 = stp.rearrange("p (b x) -> p b x", b=2)[:, :, 0:CHUNK]
            ex = spool.tile([C, 2 * CHUNK], FP8, tag="ex", bufs=EXB)
            ex3 = ex.rearrange("p (b x) -> p b x", b=2)
            if pat[s] == 0:
                nc.scalar.activation(ex3, st3, AF.Exp)
            else:
                nc.vector.tensor_scalar(out=ex3.bitcast(I8), in0=st3,
                                        scalar1=EXP8_SCALE, scalar2=EXP8_BIAS,
                                        op0=MUL, op1=ADD)
            live.setdefault("ex", {})[(ch, s)] = ex

        def emit_ou(ch, s):
            """out_u/rowsum pair s of chunk ch (consumes its ex tile)."""
            ex = live["ex"].pop((ch, s))
            ex3 = ex.rearrange("p (b x) -> p b x", b=2)
            vt3 = vt_sb[:, bass.ds(256 * s, 256)].rearrange("p (b c) -> p b c", b=2)
            nc.tensor.matmul(live["outu"], vt3, ex3, perf_mode=DR,
                             start=(s == 0), stop=(s == PAIRS - 1))
            o3 = ones_db.rearrange("p (b c) -> p b c", b=2)[:, :, 0:1]
            nc.tensor.matmul(live["rs"], o3, ex3, perf_mode=DR,
                             start=(s == 0), stop=(s == PAIRS - 1))
            if s == PAIRS - 1:
                pend.update(outu=live.pop("outu"), rs=live.pop("rs"),
                            sl=bass.ts(ch, CHUNK))

        for e in range(6):
            v_slot(e)

        for rep in range(repeats):
            for ch in range(NCHUNKS):
                first = ch == 0 and rep == 0
                pat = _chunk_pattern(ch)
                for up in range(PAIRS):
                    if up == 7 and "rs" in pend:
                        epi_a()
                    if up == 10 and "recip" in pend:
                        epi_b()
                    if first and up % 3 == 0 and 6 + up // 3 < 14:
                        v_slot(6 + up // 3)
                    if first and up in (24, 28, 32):
                        qk_chunk(1 + (up - 24) // 4)
                    if up == 12:
                        live["outu"] = ppool.tile([C, CHUNK], F32, tag="outu",
                                                  name="outu")
                        live["rs"] = ppool.tile([1, CHUNK], F32, tag="rs",
                                                name="rs")
                    emit_st(ch, up, pat, plain=(first and up < 6))
                    if not first:
                        if up < LAGP:
                            emit_ou(ch - 1, 47 + up)
                    if 12 <= up < 17:
                        emit_ou(ch, 2 * (up - 12))
                        emit_ou(ch, 2 * (up - 12) + 1)
                    elif up >= 17:
                        emit_ou(ch, up - LAGP)
            for p in range(47, PAIRS):          # final chunk's tail
                emit_ou(NCHUNKS - 1, p)
            if rep != repeats - 1:
                epi_a()
                epi_b()
                tc.strict_bb_all_engine_barrier()
        if "rs" in pend:
            epi_final()

    nc.compile()
    _BUILD_CACHE[key] = nc
    return nc


def _pack_db(a):
    """[128, X] -> DoubleRow [64, 2X] (virtual row r = p + 64*o)."""
    x = a.shape[1]
    return np.ascontiguousarray(a.reshape(2, KD, x).transpose(1, 0, 2)
                                .reshape(KD, 2 * x))


def _prep_in_maps(x_q, x_kv, Wq, bq, Wk, bk, Wv, bv, gamma):
    f32 = np.float32
    f8 = mybir.dt.np(FP8)
    x_q = np.asarray(x_q, f32).reshape(C, N)
    x_kv = np.asarray(x_kv, f32).reshape(C, N)
    Wq = np.asarray(Wq, f32)
    bq = np.asarray(bq, f32)
    Wk = np.asarray(Wk, f32)
    Wv = np.asarray(Wv, f32)
    bv = np.asarray(bv, f32)
    gamma = float(np.asarray(gamma, f32).reshape(()))

    scale = 1.0 / np.sqrt(np.float32(RC))
    xkvd = _pack_db(x_kv).astype(f8)
    wvd = _pack_db(np.ascontiguousarray(Wv.T)).astype(f8)
    # S^T = xkv^T qk with qk = scale*(Wk^T Wq xq + Wk^T bq); bk dropped
    # (softmax-invariant per-query constant).
    # qk output channel at partition j is perm[j] = j//2 + 64*(j%2), so the
    # DoubleRow remap [64,2,x] <- [128,x] is a single in-order DMA
    perm = np.arange(C) // 2 + KD * (np.arange(C) % 2)
    wblob = np.empty((C, C + 2), f32)
    wblob[:, 0:C] = (scale * (Wq.T @ Wk))[:, perm]   # lhsT [ch, c']
    wblob[:, C] = (scale * (Wk.T @ bq))[perm]
    wblob[:, C + 1] = gamma
    xkvp = np.ascontiguousarray(x_kv[perm, 0:1536]).astype(f8)
    resid_bias = (gamma * bv).astype(f32)  # softmax rows sum to 1

    in_maps = []
    for c in range(NCORES):
        xq_slice = np.ascontiguousarray(
            x_q[:, c * NQ:(c + 1) * NQ] + resid_bias[:, None], f32)
        in_maps.append({
            "xq": xq_slice, "xkvd": xkvd, "wblob": wblob, "wvd": wvd,
            "xkvp": xkvp,
        })
    return in_maps


def kernel(x_q, x_kv, Wq, bq, Wk, bk, Wv, bv, gamma):
    nc = build_nc(repeats=1)
    in_maps = _prep_in_maps(x_q, x_kv, Wq, bq, Wk, bk, Wv, bv, gamma)
    res = run_bass_kernel_spmd(nc, in_maps, list(range(NCORES)))
    out = np.concatenate([res.results[c]["y"] for c in range(NCORES)], axis=1)
    return out.reshape(1, C, D, H, W).astype(np.float32)


# revision 13
# speedup vs baseline: 1.0111x; 1.0079x over previous
"""CrossAttentionBlock Trainium2 kernel (v5).

Math (reference):
    q = Wq@xq + bq; k = Wk@xkv (bk dropped: softmax-invariant);
    S = (q^T k)/4; P = softmax_rows(S); out = (Wv@xkv + bv) @ P^T;
    y = x_q + gamma*out

Kernel strategy (8 cores, sequence-parallel over the N=13824 queries; each
core owns NQ=1728 queries against full K/V):
  * The k path never materializes: S^T = k^T q = xkv^T (Wk^T q), so the
    host folds Wqk = scale*Wk^T Wq (and bqk) and the kernel projects xq
    ONCE to qk = Wqk@xq + bqk [C, NQ] via float32r matmuls.  S^T tiles
    contract xkv (stationary, fp8 DoubleRow) against qk (moving, fp8
    DoubleRow) over the 128 channels.
  * x_kv arrives host-packed in fp8 DoubleRow layout [64, 2N]; the v^T
    projection (stationary xkv tile, moving Wv^T) runs fp8+DoubleRow;
    gamma folds into its PSUM evacuation; gamma*bv + x_q folds into the
    residual on the host.  v^T slots are interleaved into chunk 0's pair
    loop so their evacuations fill the engines' head time.
  * exp of 23.9M elem/core is the hard limit; only ScalarE (true exp,
    ~905ns/864elem) and VectorE (Schraudolph int8 bit-trick, ~1025ns) can
    read PSUM, so exp strictly alternates, with ScalarE taking the pairs
    where VectorE runs the deferred epilogue (reciprocal / normalize).
  * The pair pipeline is SEAMLESS across chunks: each chunk's loop body
    carries the previous chunk's last 7 out_u/rowsum pairs (iterations
    0-6), runs the previous chunk's epilogue at iterations 7/10 (when its
    rowsum has just completed), and starts its own out_u at iteration 12
    with a 2-per-iteration catch-up — so the single-buffered out_u/rowsum
    PSUM banks are always free exactly when needed and no engine sees a
    chunk-boundary bubble.
  * Deferred normalization: exp(S^T) feeds accumulating out_u and
    ones-row (rowsum) matmuls; per chunk 1/rowsum broadcasts via a 1->128
    ones matmul, VectorE multiplies out_u (PSUM) by it, GpSimd adds the
    residual.  The final chunk's epilogue runs as two independent
    half-width chains to shorten the drain.
  * Attention contributes O(1e-4) of the output, so fp8 quantization on
    the attention path is invisible; the residual stays fp32.
"""

import contextlib

import numpy as np

import concourse.bass as bass
import concourse.mybir as mybir
from concourse import bacc
from concourse.tile import TileContext
from concourse.bass_utils import run_bass_kernel_spmd

F32 = mybir.dt.float32
F32R = mybir.dt.float32r
FP8 = mybir.dt.float8e4
I8 = mybir.dt.int8
AF = mybir.ActivationFunctionType
DR = mybir.MatmulPerfMode.DoubleRow
ADD = mybir.AluOpType.add
MUL = mybir.AluOpType.mult

C = 128           # channels
RC = 16           # reduced (q/k) channels
KD = 64           # DoubleRow partition dim for the xkv contraction
D = H = W = 24
N = D * H * W     # 13824 tokens
NCORES = 8
NQ = N // NCORES  # 1728 queries per core
CHUNK = 432       # query chunk
NCHUNKS = NQ // CHUNK   # 4
MT = N // 128     # 108 key tiles
PAIRS = MT // 2   # 54 key-tile pairs per chunk
LAGP = 7          # steady-state out/rs lag behind exp (pairs)
EXB = 16          # ex-tile ring depth (max transient lag is 13)
QTR = N // 4      # xkv streams in 4 column-quarters

LOG2E = 1.4426950408889634
EXP8_SCALE = 8.0 * LOG2E      # e4m3: 3 mantissa bits, bias 7
EXP8_BIAS = 56.0 - 0.3        # 7*8 + Schraudolph offset

# v^T evacuation engine per slot e (0=ScalarE 1=VectorE): 8 ACT / 6 DVE
VEVAC_PAT = [0, 1, 0, 1, 0, 1, 0, 1, 0, 1, 0, 1, 0, 0]


def _chunk_pattern(ch):
    """Exp engine per pair: alternate ACT/DVE; ACT additionally takes the
    pairs where DVE runs the previous chunk's epilogue (7: reciprocal,
    11: normalize-multiply).  Chunk 0 has no pending epilogue; chunk 3
    gives ACT the final pair so DVE is free at the drain."""
    pat = [0 if s % 2 == 0 else 1 for s in range(PAIRS)]
    if ch == 0:
        pat[27] = 0
    else:
        pat[7] = 0
        pat[11] = 0
    if ch == NCHUNKS - 1:
        pat[53] = 0
    return pat


_BUILD_CACHE: dict = {}


def build_nc(repeats: int = 1):
    key = repeats
    if key in _BUILD_CACHE:
        return _BUILD_CACHE[key]

    nc = bacc.Bacc("TRN2", target_bir_lowering=False, debug=False,
                   num_devices=NCORES)
    xq = nc.dram_tensor("xq", [C, NQ], F32R, kind="ExternalInput").ap()
    xkvd = nc.dram_tensor("xkvd", [KD, 2 * N], FP8, kind="ExternalInput").ap()
    # wblob packs wqkT [C,128] + bqk [C,1] + gam [C,1] (f32 bits)
    wblob = nc.dram_tensor("wblob", [C, C + 2], F32R, kind="ExternalInput").ap()
    wvd = nc.dram_tensor("wvd", [KD, 2 * C], FP8, kind="ExternalInput").ap()
    xkvp = nc.dram_tensor("xkvp", [C, 1536 + C], FP8, kind="ExternalInput").ap()
    y = nc.dram_tensor("y", [C, NQ], F32, kind="ExternalOutput").ap()

    with TileContext(nc) as tc, contextlib.ExitStack() as ctx:
        cpool = ctx.enter_context(tc.tile_pool(name="consts", bufs=1))
        ppool = ctx.enter_context(tc.tile_pool(name="psum", bufs=1, space="PSUM"))
        spool = ctx.enter_context(tc.tile_pool(name="work", bufs=1))

        # ---- input DMAs.  HWDGE desc-gen (~630ns) and the DMA device are
        # both globally serialized, so: few DMAs, critical-path first.
        wblob_sb = cpool.tile([C, C + 2], F32R)
        nc.sync.dma_start(wblob_sb[:], wblob[:])
        wqkT_sb = wblob_sb[:, 0:C]
        bqk_sb = wblob_sb[:, C:C + 1].bitcast(F32)
        gam_sb = wblob_sb[:, C + 1:C + 2].bitcast(F32)

        xq_sb = cpool.tile([C, NQ], F32R)
        xkvd_sb = cpool.tile([KD, 2 * N], FP8)
        wvd_sb = cpool.tile([KD, 2 * C], FP8)
        xkv3 = xkvd_sb.rearrange("p (o x) -> p o x", o=2)
        xkvd3 = xkvd.rearrange("p (o x) -> p o x", o=2)

        def xkv_qtr(j):
            nc.sync.dma_start(xkv3[:, :, j * QTR:(j + 1) * QTR],
                              xkvd3[:, :, j * QTR:(j + 1) * QTR])

        xkvp_sb = cpool.tile([C, 1536 + C], FP8)
        wvp_sb = xkvp_sb[:, 1536:1536 + C]
        nc.sync.dma_start(xq_sb[:, 0:CHUNK], xq[:, 0:CHUNK])
        nc.sync.dma_start(xkvp_sb[:], xkvp[:])
        nc.sync.dma_start(wvd_sb[:], wvd[:])
        xkv_qtr(0)
        wv3 = wvd_sb.rearrange("p (o c) -> p o c", o=2)

        ones_db = cpool.tile([C, 32], FP8)   # lhsT for DoubleRow rowsum
        nc.gpsimd.memset(ones_db[:], 1.0)
        ones_row = cpool.tile([1, C], F32)   # lhsT for 1->128 broadcast matmul
        nc.gpsimd.memset(ones_row[:], 1.0)

        def slot_st():
            return ppool.tile([C, 1024], F32, tag="st", bufs=3, name="pslot")

        # ---- qk projection: qk = Wqk@xq + bqk (float32r), fp8 out; each
        # evacuated 864-col half immediately unlocks its DoubleRow remap.
        qk_sp = cpool.tile([C, NQ], FP8)
        qk_db = cpool.tile([KD, 2 * NQ], FP8)

        qk3 = qk_db.rearrange("p (o x) -> p o x", o=2)

        def qk_chunk(i):
            stq = slot_st()
            sl = bass.ts(i, CHUNK)
            nc.tensor.matmul(stq[:, 0:CHUNK], wqkT_sb, xq_sb[:, sl],
                             start=True, stop=True)
            d2 = qk_sp[:, sl]
            nc.scalar.activation(d2, stq[:, 0:CHUNK], AF.Identity, bias=bqk_sb)
            # qk_sp partition 2p+o holds channel p+64o (host-permuted Wqk),
            # so one DMA writes the DoubleRow layout directly
            nc.sync.dma_start(qk3[:, :, sl], qk_sp[:, sl])

        # warm the PE pstate with a trivial matmul before the first real one
        warm = ppool.tile([C, 512], F32, tag="outu", bufs=1, name="warm")
        nc.tensor.matmul(warm[0:C, 0:1], ones_row[:], ones_row[:, 0:1],
                         start=True, stop=True)
        qk_chunk(0)
        # remaining inputs dispatch AFTER chunk 0's qk remap (SP in-order)
        xkv_qtr(1)
        nc.sync.dma_start(xq_sb[:, CHUNK:3 * CHUNK], xq[:, CHUNK:3 * CHUNK])
        xkv_qtr(2)
        nc.sync.dma_start(xq_sb[:, 3 * CHUNK:NQ], xq[:, 3 * CHUNK:NQ])
        xkv_qtr(3)

        vt_sb = cpool.tile([C, N], FP8)

        def v_slot(e):
            """v^T projection slot e (tiles 8e..8e+7; slot 13 has 4),
            evacuated with gamma folded in, on the patterned engine."""
            psv = slot_st()
            nt = 4 if e == 13 else 8
            for j in range(nt):
                t = 8 * e + j
                if e == 0:
                    nc.tensor.matmul(psv[:, bass.ts(j, 128)],
                                     xkvp_sb[:, bass.ts(t, 128)], wvp_sb[:],
                                     start=True, stop=True)
                else:
                    nc.tensor.matmul(psv[:, bass.ts(j, 128)],
                                     xkv3[:, :, bass.ts(t, 128)], wv3[:],
                                     start=True, stop=True, perf_mode=DR)
            dst = vt_sb[:, bass.ds(1024 * e, 128 * nt)]
            src = psv[:, 0:128 * nt]
            if VEVAC_PAT[e] == 0:
                nc.scalar.activation(dst, src, AF.Identity, scale=gam_sb)
            else:
                nc.vector.tensor_scalar(out=dst, in0=src, scalar1=gam_sb,
                                        scalar2=None, op0=MUL)

        # ---- attention: one seamless pair pipeline ----------------------
        pend = {}
        live = {}   # per-chunk outu/rs psum tiles + ex ring

        def epi_a():
            pend["recip"] = recip = spool.tile([1, CHUNK], F32, tag="recip",
                                               bufs=2, name="recip")
            nc.vector.reciprocal_approx_fast(out=recip[:], in_=pend.pop("rs")[:])

        def epi_b():
            sl = pend.pop("sl")
            bcpt = ppool.tile([C, 1024], F32, tag="st", bufs=3, name="bcpt")
            bcp = bcpt[:, 0:CHUNK]
            nc.tensor.matmul(bcp, ones_row[:], pend.pop("recip")[:],
                             start=True, stop=True)
            bcs = spool.tile([C, CHUNK], F32, tag="bcs", bufs=2)
            nc.scalar.copy(bcs[:], bcp)
            t1 = spool.tile([C, CHUNK], F32, tag="t1", bufs=2)
            nc.vector.tensor_mul(t1[:], pend.pop("outu")[:], bcs[:])
            res = spool.tile([C, CHUNK], F32, tag="res", bufs=2)
            nc.gpsimd.tensor_add(res[:], t1[:], xq_sb[:, sl].bitcast(F32))
            nc.sync.dma_start(y[:, sl], res[:])

        def epi_final():
            sl0 = pend.pop("sl").start
            epi_a()
            recip = pend.pop("recip")
            outu = pend.pop("outu")
            pieces = [(0, 304), (304, 128)]
            for h, (off, w) in enumerate(pieces):
                hs = bass.ds(off, w)
                bcpt = ppool.tile([C, 1024], F32, tag="st", bufs=3, name="bcpt")
                bcp = bcpt[:, 0:w]
                nc.tensor.matmul(bcp, ones_row[:], recip[:, hs],
                                 start=True, stop=True)
                bcs = spool.tile([C, w], F32, tag="bcs2", bufs=3, name="bcs")
                nc.scalar.copy(bcs[:], bcp)
                t1 = spool.tile([C, w], F32, tag="t12", bufs=3, name="t1")
                nc.vector.tensor_mul(t1[:], outu[:, hs], bcs[:])
                res = spool.tile([C, w], F32, tag="res2", bufs=3, name="res")
                xs = xq_sb[:, bass.ds(sl0 + off, w)].bitcast(F32)
                if h == 0:
                    nc.gpsimd.tensor_add(res[:], t1[:], xs)
                else:
                    nc.vector.tensor_add(res[:], t1[:], xs)
                nc.sync.dma_start(y[:, bass.ds(sl0 + off, w)], res[:])

        def emit_st(ch, s, pat, plain=False):
            """S^T pair s of chunk ch + its exp.  plain=True uses the
            non-DoubleRow copy of the first xkv tiles so chunk 0 can start
            before the qk DoubleRow remap DMA lands."""
            stp = ppool.tile([C, 1024], F32, tag="st", bufs=3)
            for j in range(2):
                t = 2 * s + j
                if plain:
                    nc.tensor.matmul(stp[:, 512 * j:512 * j + CHUNK],
                                     xkvp_sb[:, bass.ts(t, 128)],
                                     qk_sp[:, bass.ts(ch, CHUNK)],
                                     start=True, stop=True)
                else:
                    nc.tensor.matmul(stp[:, 512 * j:512 * j + CHUNK],
                                     xkv3[:, :, bass.ts(t, 128)],
                                     qk3[:, :, bass.ts(ch, CHUNK)],
                                     start=True, stop=True, perf_mode=DR)
            st3 = stp.rearrange("p (b x) -> p b x", b=2)[:, :, 0:CHUNK]
            ex = spool.tile([C, 2 * CHUNK], FP8, tag="ex", bufs=EXB)
            ex3 = ex.rearrange("p (b x) -> p b x", b=2)
            if pat[s] == 0:
                nc.scalar.activation(ex3, st3, AF.Exp)
            else:
                nc.vector.tensor_scalar(out=ex3.bitcast(I8), in0=st3,
                                        scalar1=EXP8_SCALE, scalar2=EXP8_BIAS,
                                        op0=MUL, op1=ADD)
            live.setdefault("ex", {})[(ch, s)] = ex

        def emit_ou(ch, s):
            """out_u/rowsum pair s of chunk ch (consumes its ex tile)."""
            ex = live["ex"].pop((ch, s))
            ex3 = ex.rearrange("p (b x) -> p b x", b=2)
            vt3 = vt_sb[:, bass.ds(256 * s, 256)].rearrange("p (b c) -> p b c", b=2)
            nc.tensor.matmul(live["outu"], vt3, ex3, perf_mode=DR,
                             start=(s == 0), stop=(s == PAIRS - 1))
            o3 = ones_db.rearrange("p (b c) -> p b c", b=2)[:, :, 0:1]
            nc.tensor.matmul(live["rs"], o3, ex3, perf_mode=DR,
                             start=(s == 0), stop=(s == PAIRS - 1))
            if s == PAIRS - 1:
                pend.update(outu=live.pop("outu"), rs=live.pop("rs"),
                            sl=bass.ts(ch, CHUNK))

        for rep in range(repeats):
            for ch in range(NCHUNKS):
                first = ch == 0 and rep == 0
                pat = _chunk_pattern(ch)
                for up in range(PAIRS):
                    if up == 7 and "rs" in pend:
                        epi_a()
                    if up == 10 and "recip" in pend:
                        epi_b()
                    if first and up % 3 == 1 and up // 3 < 14:
                        v_slot(up // 3)
                    if first and up in (24, 28, 32):
                        qk_chunk(1 + (up - 24) // 4)
                    if up == 12:
                        live["outu"] = ppool.tile([C, CHUNK], F32, tag="outu",
                                                  name="outu")
                        live["rs"] = ppool.tile([1, CHUNK], F32, tag="rs",
                                                name="rs")
                    emit_st(ch, up, pat, plain=(first and up < 6))
                    if not first:
                        if up < LAGP:
                            emit_ou(ch - 1, 47 + up)
                    if 12 <= up < 17:
                        emit_ou(ch, 2 * (up - 12))
                        emit_ou(ch, 2 * (up - 12) + 1)
                    elif up >= 17:
                        emit_ou(ch, up - LAGP)
            for p in range(47, PAIRS):          # final chunk's tail
                emit_ou(NCHUNKS - 1, p)
            if rep != repeats - 1:
                epi_a()
                epi_b()
                tc.strict_bb_all_engine_barrier()
        if "rs" in pend:
            epi_final()

    nc.compile()
    _BUILD_CACHE[key] = nc
    return nc


def _pack_db(a):
    """[128, X] -> DoubleRow [64, 2X] (virtual row r = p + 64*o)."""
    x = a.shape[1]
    return np.ascontiguousarray(a.reshape(2, KD, x).transpose(1, 0, 2)
                                .reshape(KD, 2 * x))


def _prep_in_maps(x_q, x_kv, Wq, bq, Wk, bk, Wv, bv, gamma):
    f32 = np.float32
    f8 = mybir.dt.np(FP8)
    x_q = np.asarray(x_q, f32).reshape(C, N)
    x_kv = np.asarray(x_kv, f32).reshape(C, N)
    Wq = np.asarray(Wq, f32)
    bq = np.asarray(bq, f32)
    Wk = np.asarray(Wk, f32)
    Wv = np.asarray(Wv, f32)
    bv = np.asarray(bv, f32)
    gamma = float(np.asarray(gamma, f32).reshape(()))

    scale = 1.0 / np.sqrt(np.float32(RC))
    xkvd = _pack_db(x_kv).astype(f8)
    wvd = _pack_db(np.ascontiguousarray(Wv.T)).astype(f8)
    # S^T = xkv^T qk with qk = scale*(Wk^T Wq xq + Wk^T bq); bk dropped
    # (softmax-invariant per-query constant).
    # qk output channel at partition j is perm[j] = j//2 + 64*(j%2), so the
    # DoubleRow remap [64,2,x] <- [128,x] is a single in-order DMA
    perm = np.arange(C) // 2 + KD * (np.arange(C) % 2)
    wblob = np.empty((C, C + 2), f32)
    wblob[:, 0:C] = (scale * (Wq.T @ Wk))[:, perm]   # lhsT [ch, c']
    wblob[:, C] = (scale * (Wk.T @ bq))[perm]
    wblob[:, C + 1] = gamma
    xkvp = np.empty((C, 1536 + C), np.float32)
    xkvp[:, 0:1536] = x_kv[perm, 0:1536]
    xkvp[:, 1536:] = Wv.T[perm]
    xkvp = np.ascontiguousarray(xkvp).astype(f8)
    resid_bias = (gamma * bv).astype(f32)  # softmax rows sum to 1

    in_maps = []
    for c in range(NCORES):
        xq_slice = np.ascontiguousarray(
            x_q[:, c * NQ:(c + 1) * NQ] + resid_bias[:, None], f32)
        in_maps.append({
            "xq": xq_slice, "xkvd": xkvd, "wblob": wblob, "wvd": wvd,
            "xkvp": xkvp,
        })
    return in_maps


def kernel(x_q, x_kv, Wq, bq, Wk, bk, Wv, bv, gamma):
    nc = build_nc(repeats=1)
    in_maps = _prep_in_maps(x_q, x_kv, Wq, bq, Wk, bk, Wv, bv, gamma)
    res = run_bass_kernel_spmd(nc, in_maps, list(range(NCORES)))
    out = np.concatenate([res.results[c]["y"] for c in range(NCORES)], axis=1)
    return out.reshape(1, C, D, H, W).astype(np.float32)
